# revision 1
# baseline (speedup 1.0000x reference)
"""Multi-head attention kernel for Trainium2 (Bass/Tile), 8-core SPMD.

Problem: Q,K,V [B=2, H=16, S=4096, D=64] fp32 -> softmax(Q K^T / sqrt(D)) V.
Sharding: batch*heads (32) split 4-per-core across 8 NeuronCores; each core
computes its heads independently (no collectives).

Per-head algorithm (transposed-scores flash attention, no max subtraction --
inputs are unit-normal so scores are O(6) and exp never overflows fp32):
  scoresT[k,q] = K[k,:] . Q[q,:]        (PE, fp32r, row-tiled pairs)
  pT[k,q]     = exp(scoresT / 8)        (ACT, scale fused into activation)
  accT[d,q]  += Vaug[k,d] . pT[k,q]     (PE, fp32r; Vaug row 64 == ones, so
                                         acc row 64 accumulates the softmax
                                         denominator)
  out[q,d]    = accT[d,q] / accT[64,q]  (PE transpose + DVE recip/scale)

Host side only re-lays-out data: QT/KT transposed per head, V augmented with
a ones column, output gathered. All arithmetic runs on device.
"""

import numpy as np
from contextlib import ExitStack

import concourse.bacc as bacc
import concourse.bass as bass
import concourse.tile as tile
import concourse.mybir as mybir
from concourse.bass_utils import run_bass_kernel_spmd

F32 = mybir.dt.float32
F32R = mybir.dt.float32r
EXP = mybir.ActivationFunctionType.Exp

B, H, S, D = 2, 16, 4096, 64
N_CORES = 8
HPC = (B * H) // N_CORES  # heads per core

QTILE = 512            # q columns processed per inner iteration
CHUNK = 128            # k rows per matmul (PE partition dim)
GROUP = 2              # k-chunks exp'd per ACT instruction


def build_nc(hpc: int = HPC, s: int = S, qtile: int = QTILE):
    n_chunks = s // CHUNK
    n_qtiles = s // qtile
    n_groups = n_chunks // GROUP

    nc = bacc.Bacc("TRN2", target_bir_lowering=False, debug=False)
    qt_d = nc.dram_tensor("qt", [hpc, D, s], F32R, kind="ExternalInput").ap()
    kt_d = nc.dram_tensor("kt", [hpc, D, s], F32R, kind="ExternalInput").ap()
    va_d = nc.dram_tensor("va", [hpc, s, D + 1], F32R, kind="ExternalInput").ap()
    id_d = nc.dram_tensor("ident", [128, 128], F32, kind="ExternalInput").ap()
    o_d = nc.dram_tensor("o", [hpc, s, D], F32, kind="ExternalOutput").ap()

    with tile.TileContext(nc) as tc, ExitStack() as ctx:
        qk_pool = ctx.enter_context(tc.tile_pool(name="qk", bufs=2))
        v_pool = ctx.enter_context(tc.tile_pool(name="v", bufs=2))
        pt_pool = ctx.enter_context(tc.tile_pool(name="pt", bufs=2))
        ot_pool = ctx.enter_context(tc.tile_pool(name="ot", bufs=2))
        oacc_pool = ctx.enter_context(tc.tile_pool(name="oacc", bufs=2))
        small_pool = ctx.enter_context(tc.tile_pool(name="small", bufs=4))
        const_pool = ctx.enter_context(tc.tile_pool(name="const", bufs=1))
        sc_psum = ctx.enter_context(tc.tile_pool(name="sc", bufs=2, space="PSUM"))
        oa_psum = ctx.enter_context(tc.tile_pool(name="oa", bufs=2, space="PSUM"))
        tp_psum = ctx.enter_context(tc.tile_pool(name="tp", bufs=2, space="PSUM"))

        ident = const_pool.tile([128, 128], F32)
        nc.sync.dma_start(ident[:], id_d)

        for h in range(hpc):
            # K^T and Q^T [D, s] duplicated into both partition halves so two
            # k-chunks can run concurrently via PE row tiling (rows 0-63 and
            # 64-127 each see contraction depth D=64).
            qt_sb = qk_pool.tile([128, s], F32R, tag="qt")
            kt_sb = qk_pool.tile([128, s], F32R, tag="kt")
            nc.sync.dma_start(qt_sb[0:D, :], qt_d[h])
            nc.sync.dma_start(qt_sb[D : 2 * D, :], qt_d[h])
            nc.sync.dma_start(kt_sb[0:D, :], kt_d[h])
            nc.sync.dma_start(kt_sb[D : 2 * D, :], kt_d[h])
            va_sb = v_pool.tile([128, n_chunks, D + 1], F32R)
            nc.sync.dma_start(
                va_sb[:], va_d[h].rearrange("(c p) e -> p c e", p=128)
            )
            o_acc = oacc_pool.tile([128, s // 128, D], F32)

            for qt in range(n_qtiles):
                qs = slice(qt * qtile, (qt + 1) * qtile)
                acc = oa_psum.tile([D + 1, qtile], F32)
                for g in range(n_groups):
                    ca, cb = 2 * g, 2 * g + 1
                    sc = sc_psum.tile([128, 2 * qtile], F32)
                    nc.tensor.matmul(
                        sc[:, 0:qtile],
                        kt_sb[0:D, ca * CHUNK : (ca + 1) * CHUNK],
                        qt_sb[0:D, qs],
                        start=True, stop=True,
                    )
                    nc.tensor.matmul(
                        sc[:, qtile : 2 * qtile],
                        kt_sb[D : 2 * D, cb * CHUNK : (cb + 1) * CHUNK],
                        qt_sb[D : 2 * D, qs],
                        start=True, stop=True,
                    )
                    p_t = pt_pool.tile([128, 2 * qtile], F32R)
                    nc.scalar.activation(p_t[:], sc[:], EXP, scale=float(1.0 / np.sqrt(D)))
                    nc.tensor.matmul(
                        acc[:], va_sb[:, ca, :], p_t[:, 0:qtile],
                        start=(g == 0), stop=False,
                    )
                    nc.tensor.matmul(
                        acc[:], va_sb[:, cb, :], p_t[:, qtile : 2 * qtile],
                        start=False, stop=(g == n_groups - 1),
                    )

                # epilogue: normalize + transpose back to [q, D]
                ot = ot_pool.tile([D + 1, qtile], F32)
                nc.vector.tensor_copy(ot[:], acc[:])
                for sub in range(qtile // 128):
                    tp = tp_psum.tile([128, D + 1], F32)
                    nc.tensor.transpose(
                        tp[:], ot[:, sub * 128 : (sub + 1) * 128],
                        ident[0 : D + 1, 0 : D + 1],
                    )
                    dinv = small_pool.tile([128, 1], F32)
                    nc.vector.reciprocal(dinv[:], tp[:, D : D + 1])
                    nc.vector.tensor_scalar_mul(
                        o_acc[:, qt * (qtile // 128) + sub, :], tp[:, 0:D], dinv[:]
                    )

            nc.sync.dma_start(
                o_d[h].rearrange("(c p) d -> p c d", p=128), o_acc[:]
            )

    nc.compile()
    return nc


_NC_CACHE = {}


def _get_nc(hpc=HPC, s=S, qtile=QTILE):
    key = (hpc, s, qtile)
    if key not in _NC_CACHE:
        _NC_CACHE[key] = build_nc(hpc, s, qtile)
    return _NC_CACHE[key]


def prep_inputs(Q, K, V):
    """Host-side re-layout: per-core input maps."""
    bh = B * H
    q2 = np.ascontiguousarray(
        np.asarray(Q, dtype=np.float32).reshape(bh, S, D).transpose(0, 2, 1)
    )
    k2 = np.ascontiguousarray(
        np.asarray(K, dtype=np.float32).reshape(bh, S, D).transpose(0, 2, 1)
    )
    v = np.asarray(V, dtype=np.float32).reshape(bh, S, D)
    va = np.concatenate([v, np.ones((bh, S, 1), dtype=np.float32)], axis=-1)
    ident = np.eye(128, dtype=np.float32)
    in_maps = []
    for c in range(N_CORES):
        sl = slice(c * HPC, (c + 1) * HPC)
        in_maps.append({
            "qt": np.ascontiguousarray(q2[sl]),
            "kt": np.ascontiguousarray(k2[sl]),
            "va": np.ascontiguousarray(va[sl]),
            "ident": ident,
        })
    return in_maps


def run(Q, K, V, trace=False, **kwargs):
    nc = _get_nc()
    in_maps = prep_inputs(Q, K, V)
    res = run_bass_kernel_spmd(
        nc, in_maps, core_ids=list(range(N_CORES)), trace=trace, **kwargs
    )
    outs = [res.results[c]["o"] for c in range(N_CORES)]
    full = np.concatenate(outs, axis=0).reshape(B, H, S, D)
    return full, res


def kernel(Q, K, V):
    out, _ = run(Q, K, V)
    return out


# revision 7
# speedup vs baseline: 1.1205x; 1.1205x over previous
"""Multi-head attention kernel for Trainium2 (Bass/Tile), 8-core SPMD.

Problem: Q,K,V [B=2, H=16, S=4096, D=64] fp32 -> softmax(Q K^T / sqrt(D)) V.
Sharding: batch*heads (32) split 4-per-core across 8 NeuronCores; each core
computes its heads independently (no collectives).

Per-head algorithm (transposed-scores flash attention, no max subtraction --
inputs are unit-normal so scores are O(6) and exp never overflows fp32):
  scoresT[k,q] = K[k,:] . Q[q,:]        (PE, fp32r, row-tiled pairs)
  pT[k,q]     = exp(scoresT / 8)        (ACT, scale fused into activation)
  accT[d,q]  += Vaug[k,d] . pT[k,q]     (PE, fp32r; Vaug row 64 == ones, so
                                         acc row 64 accumulates the softmax
                                         denominator)
  out[q,d]    = accT[d,q] / accT[64,q]  (PE transpose + DVE recip/scale)

Host side only re-lays-out data: QT/KT transposed per head, V augmented with
a ones column, output gathered. All arithmetic runs on device.
"""

import numpy as np
from contextlib import ExitStack

import concourse.bacc as bacc
import concourse.bass as bass
import concourse.tile as tile
import concourse.mybir as mybir
from concourse.bass_utils import run_bass_kernel_spmd

F32 = mybir.dt.float32
F16 = mybir.dt.float16
EXP = mybir.ActivationFunctionType.Exp

B, H, S, D = 2, 16, 4096, 64
N_CORES = 8
HPC = (B * H) // N_CORES  # heads per core

QTILE = 512            # q columns processed per inner iteration
CHUNK = 128            # k rows per matmul (PE partition dim)
GROUP = 2              # k-chunks exp'd per ACT instruction


def build_nc(hpc: int = HPC, s: int = S, qtile: int = QTILE):
    n_chunks = s // CHUNK
    n_qtiles = s // qtile
    n_groups = n_chunks // GROUP

    nc = bacc.Bacc("TRN2", target_bir_lowering=False, debug=False)
    qt_d = nc.dram_tensor("qt", [hpc, D, s], F16, kind="ExternalInput").ap()
    kt_d = nc.dram_tensor("kt", [hpc, D, s], F16, kind="ExternalInput").ap()
    va_d = nc.dram_tensor("va", [hpc, s, D + 1], F16, kind="ExternalInput").ap()
    id_d = nc.dram_tensor("ident", [128, 128], F32, kind="ExternalInput").ap()
    o_d = nc.dram_tensor("o", [hpc, s, D], F32, kind="ExternalOutput").ap()

    with tile.TileContext(nc) as tc, ExitStack() as ctx:
        qk_pool = ctx.enter_context(tc.tile_pool(name="qk", bufs=2))
        v_pool = ctx.enter_context(tc.tile_pool(name="v", bufs=2))
        pt_pool = ctx.enter_context(tc.tile_pool(name="pt", bufs=2))
        ot_pool = ctx.enter_context(tc.tile_pool(name="ot", bufs=2))
        oacc_pool = ctx.enter_context(tc.tile_pool(name="oacc", bufs=2))
        small_pool = ctx.enter_context(tc.tile_pool(name="small", bufs=4))
        const_pool = ctx.enter_context(tc.tile_pool(name="const", bufs=1))
        sc_psum = ctx.enter_context(tc.tile_pool(name="sc", bufs=2, space="PSUM"))
        oa_psum = ctx.enter_context(tc.tile_pool(name="oa", bufs=2, space="PSUM"))
        tp_psum = ctx.enter_context(tc.tile_pool(name="tp", bufs=2, space="PSUM"))

        ident = const_pool.tile([128, 128], F32)
        nc.sync.dma_start(ident[:], id_d)

        for h in range(hpc):
            # K^T and Q^T [D, s] duplicated into both partition halves so two
            # k-chunks can run concurrently via PE row tiling (rows 0-63 and
            # 64-127 each see contraction depth D=64).
            qt_sb = qk_pool.tile([128, s], F16, tag="qt")
            kt_sb = qk_pool.tile([128, s], F16, tag="kt")
            nc.sync.dma_start(qt_sb[0:D, :], qt_d[h])
            nc.sync.dma_start(qt_sb[D : 2 * D, :], qt_d[h])
            nc.sync.dma_start(kt_sb[0:D, :], kt_d[h])
            nc.sync.dma_start(kt_sb[D : 2 * D, :], kt_d[h])
            va_sb = v_pool.tile([128, n_chunks, D + 1], F16)
            nc.sync.dma_start(
                va_sb[:], va_d[h].rearrange("(c p) e -> p c e", p=128)
            )
            o_acc = oacc_pool.tile([128, s // 128, D], F32)

            for qt in range(n_qtiles):
                qs = slice(qt * qtile, (qt + 1) * qtile)
                acc = oa_psum.tile([D + 1, qtile], F32)
                for g in range(n_groups):
                    ca, cb = 2 * g, 2 * g + 1
                    sc = sc_psum.tile([128, 2 * qtile], F32)
                    nc.tensor.matmul(
                        sc[:, 0:qtile],
                        kt_sb[0:D, ca * CHUNK : (ca + 1) * CHUNK],
                        qt_sb[0:D, qs],
                        start=True, stop=True,
                    )
                    nc.tensor.matmul(
                        sc[:, qtile : 2 * qtile],
                        kt_sb[D : 2 * D, cb * CHUNK : (cb + 1) * CHUNK],
                        qt_sb[D : 2 * D, qs],
                        start=True, stop=True,
                    )
                    p_t = pt_pool.tile([128, 2 * qtile], F16)
                    nc.scalar.activation(p_t[:], sc[:], EXP, scale=float(1.0 / np.sqrt(D)))
                    nc.tensor.matmul(
                        acc[:], va_sb[:, ca, :], p_t[:, 0:qtile],
                        start=(g == 0), stop=False,
                    )
                    nc.tensor.matmul(
                        acc[:], va_sb[:, cb, :], p_t[:, qtile : 2 * qtile],
                        start=False, stop=(g == n_groups - 1),
                    )

                # epilogue: normalize + transpose back to [q, D]
                ot = ot_pool.tile([D + 1, qtile], F32)
                nc.vector.tensor_copy(ot[:], acc[:])
                for sub in range(qtile // 128):
                    tp = tp_psum.tile([128, D + 1], F32)
                    nc.tensor.transpose(
                        tp[:], ot[:, sub * 128 : (sub + 1) * 128],
                        ident[0 : D + 1, 0 : D + 1],
                    )
                    dinv = small_pool.tile([128, 1], F32)
                    nc.vector.reciprocal(dinv[:], tp[:, D : D + 1])
                    nc.vector.tensor_scalar_mul(
                        o_acc[:, qt * (qtile // 128) + sub, :], tp[:, 0:D], dinv[:]
                    )

            nc.sync.dma_start(
                o_d[h].rearrange("(c p) d -> p c d", p=128), o_acc[:]
            )

    nc.compile()
    return nc


_NC_CACHE = {}


def _get_nc(hpc=HPC, s=S, qtile=QTILE):
    key = (hpc, s, qtile)
    if key not in _NC_CACHE:
        _NC_CACHE[key] = build_nc(hpc, s, qtile)
    return _NC_CACHE[key]


def prep_inputs(Q, K, V):
    """Host-side re-layout: per-core input maps."""
    bh = B * H
    q2 = np.ascontiguousarray(
        np.asarray(Q, dtype=np.float32).reshape(bh, S, D).transpose(0, 2, 1)
    ).astype(np.float16)
    k2 = np.ascontiguousarray(
        np.asarray(K, dtype=np.float32).reshape(bh, S, D).transpose(0, 2, 1)
    ).astype(np.float16)
    v = np.asarray(V, dtype=np.float32).reshape(bh, S, D).astype(np.float16)
    va = np.concatenate([v, np.ones((bh, S, 1), dtype=np.float16)], axis=-1)
    ident = np.eye(128, dtype=np.float32)
    in_maps = []
    for c in range(N_CORES):
        sl = slice(c * HPC, (c + 1) * HPC)
        in_maps.append({
            "qt": np.ascontiguousarray(q2[sl]),
            "kt": np.ascontiguousarray(k2[sl]),
            "va": np.ascontiguousarray(va[sl]),
            "ident": ident,
        })
    return in_maps


def run(Q, K, V, trace=False, **kwargs):
    nc = _get_nc()
    in_maps = prep_inputs(Q, K, V)
    res = run_bass_kernel_spmd(
        nc, in_maps, core_ids=list(range(N_CORES)), trace=trace, **kwargs
    )
    outs = [res.results[c]["o"] for c in range(N_CORES)]
    full = np.concatenate(outs, axis=0).reshape(B, H, S, D)
    return full, res


def kernel(Q, K, V):
    out, _ = run(Q, K, V)
    return out
